# revision 15
# baseline (speedup 1.0000x reference)
"""GCN (2-layer GCNConv + log_softmax) on 8 Trainium2 NeuronCores — v2.

Scatter (src-partitioned) formulation:
  - Nodes rebalanced into 392 blocks of 128 by in-degree (greedy), padded to
    N_PAD = 50176; shard c owns nodes [c*6272, (c+1)*6272).
  - Edges (excluding the added self-loops) are owned by the src's core,
    grouped by global dst block, padded to T2 tiles of 128 per (core,
    block). Normalization factorizes: tables store h' = dinv*h, the
    selector is a pure 0/1 one-hot, dst-side dinv is applied after reduce.
    The self-loop diagonal is a per-block identity matmul masked by a
    per-core input (I for own blocks, 0 otherwise) appended to each
    block's accumulation chain.
  - Per core: GEMM1 -> h' (resident + local HBM table). For each of 7
    chunks (56 dst blocks): batched dma_gather of h'[src] (local table,
    int16 idx), one-hot selector via is_equal, segment-sum via PE matmul
    accumulation, partial [hid, dst] written to a per-chunk partial
    table, then ReduceScatter(add) delivers each core its summed dst
    rows; chunked RS overlaps the next chunk's compute. Post-RS:
    relu(dinv_d*agg + b1), GEMM2 -> h2' table. Layer 2 repeats the same
    structure (same gather indices) producing logits partials; post-RS:
    bias, log_softmax, store output shard.
  - Host un-permutes and strips padding.
"""

import math

import numpy as np
import ml_dtypes

P = 128
NC = 8
NCORES = NC = 8

N_NODES = 50000
N_EDGES = 800000
F_IN = 512
HID = 128
NCLS = 40

SH = 6272            # nodes per shard
NPAD = NC * SH       # 50176
NBLK = 392           # global dst blocks of 128
NBPS = 49            # blocks per shard
KCH = 7              # chunks
BPC = 7              # blocks per chunk per shard
SLOTS = NC * BPC     # 56 dst blocks per chunk
GR = 2               # blocks per dma_gather (dma_gather num_idxs limit: 1024)
NG = SLOTS // GR     # 4 gathers per chunk
CH_ROWS = BPC * P    # 896 rows per chunk per shard
NCLS_PAD = 128

# runtime knobs for test.py
TRACE = False
TRACE_KWARGS = {}
PART_DT = "float16"    # layer-1 partial-table dtype for ReduceScatter
PART2_DT = "float16"   # layer-2 partial-table dtype
TAB_DT = "float16"     # gather tables / GEMM operands / selector dtype
PHASE = 6              # build stages 1..6 (debug bisect)

LAST_RESULT = {}

_bf16 = ml_dtypes.bfloat16


def _tab_np():
    return {"float16": __import__("numpy").float16, "float32": __import__("numpy").float32,
            "bfloat16": ml_dtypes.bfloat16}[TAB_DT]


# --------------------------------------------------------------------------
# Host preprocessing
# --------------------------------------------------------------------------

def _balance_blocks(dst, n_nodes):
    """Greedy assign nodes to 392 blocks of <=128 balancing in-degree."""
    import heapq
    w = np.bincount(dst, minlength=n_nodes)
    order = np.argsort(-w, kind="stable")
    heap = [(0, b) for b in range(NBLK)]
    heapq.heapify(heap)
    cap = np.full(NBLK, P, np.int64)
    sums = np.zeros(NBLK, np.int64)
    slot = np.zeros(NBLK, np.int64)
    newid = np.empty(n_nodes, np.int64)
    for v in order:
        while True:
            _, b = heapq.heappop(heap)
            if cap[b] > 0:
                break
        cap[b] -= 1
        sums[b] += w[v]
        newid[v] = b * P + slot[b]
        slot[b] += 1
        heapq.heappush(heap, (int(sums[b]), b))
    return newid


def _preprocess(edge_index):
    """Returns (newid, dinv, idx_all, seg_all, T2)."""
    src = np.asarray(edge_index[0], dtype=np.int64)
    dst = np.asarray(edge_index[1], dtype=np.int64)
    newid = _balance_blocks(dst, N_NODES)

    ns = newid[src]
    nt = newid[dst]

    deg = np.bincount(nt, minlength=NPAD).astype(np.float64) + 1.0
    dinv = (1.0 / np.sqrt(deg)).astype(np.float32)

    core = ns // SH
    B = nt // P                      # global dst block
    cprime = B // NBPS
    bis = B % NBPS
    k = bis // BPC
    blk = bis % BPC
    slot = cprime * BPC + blk        # 0..55

    cell = (core * KCH + k) * SLOTS + slot
    cnt = np.bincount(cell, minlength=NC * KCH * SLOTS)
    T2 = max(1, int(math.ceil(cnt.max() / P)))

    nidx = GR * T2 * P               # idx per gather
    icols = nidx // 16
    ntile = NBLK * T2                # tiles per core
    gath = KCH * NG                  # gathers per core per layer

    idx_all = np.full((NC, P, gath * icols), SH, np.int16)  # pad -> zero row
    seg_all = np.zeros((NC, P, ntile), np.float32)

    order = np.lexsort((np.arange(len(ns)), cell))
    s_cell = cell[order]
    s_src = ns[order]
    s_d = (nt[order] % P).astype(np.float32)
    starts = np.searchsorted(s_cell, np.arange(NC * KCH * SLOTS))
    j = np.arange(len(ns)) - starts[s_cell]

    c_a = s_cell // (KCH * SLOTS)
    k_a = (s_cell // SLOTS) % KCH
    slot_a = s_cell % SLOTS
    g_a = slot_a // GR
    m_a = slot_a % GR
    t_a = j // P
    p_a = j % P
    assert (t_a < T2).all()
    i_a = m_a * (T2 * P) + t_a * P + p_a
    part = i_a % 16
    col = (k_a * NG + g_a) * icols + i_a // 16
    nloc = s_src - c_a * SH
    bis_s = nloc // P
    # table row (j' order): (bis//BPC)*CH_ROWS + (n%P)*BPC + bis%BPC
    jp = (bis_s // BPC) * CH_ROWS + (nloc % P) * BPC + (bis_s % BPC)
    idx_all[c_a, part, col] = jp.astype(np.int16)
    # the 8 GpSimd Q7 cores each read their own 16-partition stripe of the
    # index tile: replicate the [0:16] stripe across all 128 partitions
    idx_all[:, 16:, :] = np.tile(idx_all[:, :16, :], (1, 7, 1))
    tile_idx = k_a * (SLOTS * T2) + slot_a * T2 + t_a
    seg_all[c_a, p_a, tile_idx] = s_d

    return newid, dinv, idx_all, seg_all, T2


# --------------------------------------------------------------------------
# Device program
# --------------------------------------------------------------------------

def _np_pdt(name):
    return {"float16": np.float16, "float32": np.float32,
            "bfloat16": _bf16}[name]


def _build_program(T2):
    import concourse.bacc as bacc
    import concourse.mybir as mybir
    import concourse.tile as tile

    dt = mybir.dt
    f32 = dt.float32
    name2dt = {"float16": dt.float16, "float32": dt.float32,
               "bfloat16": dt.bfloat16}
    bf16 = name2dt[TAB_DT]
    p1dt = name2dt[PART_DT]
    p2dt = name2dt[PART2_DT]

    nidx = GR * T2 * P
    icols = nidx // 16
    ntile = NBLK * T2
    gath = KCH * NG
    gt = GR * T2                     # tiles per gather (42)
    kt = F_IN // P                   # 4

    nc = bacc.Bacc(
        "TRN2",
        target_bir_lowering=False,
        debug=False,
        enable_asserts=False,
        num_devices=NC,
    )

    # ---- kernel I/O ----
    xt_d = nc.dram_tensor("xt", [F_IN, SH], bf16, kind="ExternalInput")
    w1_d = nc.dram_tensor("w1", [P, kt * HID], bf16, kind="ExternalInput")
    w2_d = nc.dram_tensor("w2", [HID, NCLS_PAD], bf16, kind="ExternalInput")
    b1_d = nc.dram_tensor("b1", [P, 1], f32, kind="ExternalInput")
    b2rep_d = nc.dram_tensor("b2rep", [P, BPC * NCLS], f32, kind="ExternalInput")
    dinvn_d = nc.dram_tensor("dinvn", [P, NBPS], f32, kind="ExternalInput")
    dinvrow_d = nc.dram_tensor("dinvrow", [P, SH], p1dt, kind="ExternalInput")
    dinvrep2_d = nc.dram_tensor("dinvrep2", [P, NBPS * NCLS], p2dt,
                                kind="ExternalInput")
    eyem_d = nc.dram_tensor("eyem", [P, NC * P], bf16, kind="ExternalInput")
    iotaw_d = nc.dram_tensor("iotaw", [P, gt * P], bf16, kind="ExternalInput")
    seg_d = nc.dram_tensor("seg", [P, ntile], bf16, kind="ExternalInput")
    idx_d = nc.dram_tensor("idx", [P, gath * icols], dt.int16,
                           kind="ExternalInput")
    out_d = nc.dram_tensor("out", [SH, NCLS], f32, kind="ExternalOutput")

    RG = [list(range(NC))]

    with tile.TileContext(nc) as tc:
        with (
            tc.tile_pool(name="const", bufs=1) as const,
            tc.tile_pool(name="dram", bufs=1, space="DRAM") as dram,
            tc.tile_pool(name="sb", bufs=2) as sb,
            tc.tile_pool(name="psum", bufs=2, space="PSUM") as psum,
        ):
            # ---- DRAM scratch ----
            h1tab = dram.tile([SH + P, P], bf16, tag="h1tab")
            h2tab = dram.tile([SH + P, P], bf16, tag="h2tab")
            tab1 = [dram.tile([NC, P, CH_ROWS], p1dt, name=f"tab1_{k}")
                    for k in range(KCH)]
            agg1 = [dram.tile([P, CH_ROWS], p1dt, name=f"agg1_{k}")
                    for k in range(KCH)]
            tab2 = [dram.tile([NC, CH_ROWS, NCLS], p2dt, name=f"tab2_{k}")
                    for k in range(KCH)]
            out2 = [dram.tile([CH_ROWS, NCLS], p2dt, name=f"out2_{k}")
                    for k in range(KCH)]

            # ---- constants to SBUF ----
            w1_sb = const.tile([P, kt * HID], bf16)
            nc.sync.dma_start(out=w1_sb[:], in_=w1_d[:])
            w2_sb = const.tile([HID, NCLS_PAD], bf16)
            nc.sync.dma_start(out=w2_sb[:], in_=w2_d[:])
            b1_sb = const.tile([P, 1], f32)
            nc.sync.dma_start(out=b1_sb[:], in_=b1_d[:])
            b2rep_sb = const.tile([P, BPC * NCLS], f32)
            nc.sync.dma_start(out=b2rep_sb[:], in_=b2rep_d[:])
            dinvn_sb = const.tile([P, NBPS], f32)
            nc.sync.dma_start(out=dinvn_sb[:], in_=dinvn_d[:])
            dinvrow_sb = const.tile([P, SH], p1dt)
            nc.sync.dma_start(out=dinvrow_sb[:], in_=dinvrow_d[:])
            dinvrep2_sb = const.tile([P, NBPS * NCLS], p2dt)
            nc.sync.dma_start(out=dinvrep2_sb[:], in_=dinvrep2_d[:])
            eyem_sb = const.tile([P, NC * P], bf16)
            nc.sync.dma_start(out=eyem_sb[:], in_=eyem_d[:])
            iotaw_sb = const.tile([P, gt * P], bf16)
            nc.sync.dma_start(out=iotaw_sb[:], in_=iotaw_d[:])
            seg_sb = const.tile([P, ntile], bf16)
            nc.sync.dma_start(out=seg_sb[:], in_=seg_d[:])
            idx_sb = const.tile([P, gath * icols], dt.int16)
            nc.sync.dma_start(out=idx_sb[:], in_=idx_d[:])

            # resident h' and h2' (feature-transposed blocks, cols (bis, h))
            hres = const.tile([P, NBPS * P], bf16)
            h2res = const.tile([P, NBPS * P], bf16)

            # zero the pad row(s) of both tables
            ztile = const.tile([P, P], bf16)
            nc.vector.memset(ztile[:], 0.0)
            nc.scalar.dma_start(out=h1tab[SH:SH + P, :], in_=ztile[:])
            nc.scalar.dma_start(out=h2tab[SH:SH + P, :], in_=ztile[:])

            # ---------------- GEMM1: h' = dinv * (x @ W1) ----------------
            for k in range(KCH):
                xt_t = []
                for kk in range(kt):
                    t = sb.tile([P, CH_ROWS], bf16, tag=f"xt{kk}", bufs=2,
                                name=f"xt_t{kk}")
                    nc.sync.dma_start(
                        out=t[:],
                        in_=xt_d[kk * P:(kk + 1) * P,
                                 k * CH_ROWS:(k + 1) * CH_ROWS],
                    )
                    xt_t.append(t)
                for blk in range(BPC):
                    bis = k * BPC + blk
                    psum_h = psum.tile([P, HID], f32, tag="psum_h", bufs=2)
                    for kk in range(kt):
                        nc.tensor.matmul(
                            out=psum_h[:],
                            lhsT=xt_t[kk][:, blk * P:(blk + 1) * P],
                            rhs=w1_sb[:, kk * HID:(kk + 1) * HID],
                            start=(kk == 0),
                            stop=(kk == kt - 1),
                        )
                    nc.vector.tensor_scalar_mul(
                        out=hres[:, bis * P:(bis + 1) * P],
                        in0=psum_h[:],
                        scalar1=dinvn_sb[:, bis:bis + 1],
                    )
                # rows j' = d*7 + blk  <- cols (blk, h)
                nc.scalar.dma_start(
                    out=h1tab[k * CH_ROWS:(k + 1) * CH_ROWS, :].rearrange(
                        "(d b) h -> d (b h)", b=BPC),
                    in_=hres[:, k * CH_ROWS:(k + 1) * CH_ROWS],
                )

            def agg_layer(tabk, outk, res_sb, tab_is_l2):
                """One aggregation layer: gathers + selector matmuls + diag
                matmul + partial stores + chunked RS."""
                srctab = h2tab if tab_is_l2 else h1tab
                for k in range(KCH):
                    if tab_is_l2:
                        stag = sb.tile([P, SLOTS * NCLS], p2dt, tag="stag2",
                                       bufs=2, name="stag2")
                    else:
                        stag = sb.tile([P, SLOTS * P], p1dt, tag="stag1",
                                       bufs=2, name="stag1")
                    for g in range(NG):
                        msg = sb.tile([P, gt * P], bf16, tag="msg1", bufs=2,
                                      name="msg")
                        msg3 = msg.rearrange("p (t e) -> p t e", e=P)
                        nc.gpsimd.dma_gather(
                            msg3,
                            srctab[:],
                            idx_sb[:, (k * NG + g) * icols:
                                   (k * NG + g + 1) * icols],
                            nidx,
                            nidx,
                            P,
                        )
                        sel = sb.tile([P, gt * P], bf16, tag="sel1", bufs=2,
                                      name="sel")
                        sel3 = sel.rearrange("p (t d) -> p t d", d=P)
                        g0 = k * (SLOTS * T2) + g * gt
                        nc.vector.tensor_tensor(
                            out=sel3,
                            in0=iotaw_sb[:].rearrange("p (t d) -> p t d", d=P),
                            in1=seg_sb[:, g0:g0 + gt].to_broadcast([P, gt, P]),
                            op=mybir.AluOpType.is_equal,
                        )
                        for m in range(GR):
                            slot = g * GR + m
                            cp, blk = divmod(slot, BPC)
                            bis = k * BPC + blk
                            if tab_is_l2:
                                ps = psum.tile([P, NCLS], f32, tag="psum_o",
                                               bufs=2, name="ps_o")
                                for t in range(T2):
                                    nc.tensor.matmul(
                                        out=ps[:],
                                        lhsT=sel3[:, m * T2 + t, :],
                                        rhs=msg3[:, m * T2 + t, 0:NCLS],
                                        start=(t == 0),
                                        stop=False,
                                    )
                                nc.tensor.matmul(
                                    out=ps[:],
                                    lhsT=eyem_sb[:, cp * P:(cp + 1) * P],
                                    rhs=res_sb[:, bis * P:bis * P + NCLS],
                                    start=False,
                                    stop=True,
                                )
                                nc.vector.tensor_copy(
                                    out=stag[:, slot * NCLS:(slot + 1) * NCLS],
                                    in_=ps[:],
                                )
                            else:
                                ps = psum.tile([P, P], f32, tag="psum_1",
                                               bufs=2, name="ps_1")
                                for t in range(T2):
                                    nc.tensor.matmul(
                                        out=ps[:],
                                        lhsT=msg3[:, m * T2 + t, :],
                                        rhs=sel3[:, m * T2 + t, :],
                                        start=(t == 0),
                                        stop=False,
                                    )
                                nc.tensor.matmul(
                                    out=ps[:],
                                    lhsT=res_sb[:, bis * P:(bis + 1) * P],
                                    rhs=eyem_sb[:, cp * P:(cp + 1) * P],
                                    start=False,
                                    stop=True,
                                )
                                nc.vector.tensor_copy(
                                    out=stag[:, slot * P:(slot + 1) * P],
                                    in_=ps[:],
                                )
                    # store partials, alternating HWDGE queues
                    for cp in range(NC):
                        if tab_is_l2:
                            o = tabk[k][cp, :, :].rearrange(
                                "(d b) n -> d b n", b=BPC)
                            i = stag[:, cp * BPC * NCLS:
                                     (cp + 1) * BPC * NCLS].rearrange(
                                "d (b n) -> d b n", b=BPC)
                        else:
                            o = tabk[k][cp, :, :]
                            i = stag[:, cp * CH_ROWS:(cp + 1) * CH_ROWS]
                        eng = nc.sync if cp % 2 == 0 else nc.scalar
                        eng.dma_start(out=o, in_=i)
                    if (PHASE >= 3 and not tab_is_l2) or (PHASE >= 6 and tab_is_l2):
                        nc.gpsimd.collective_compute(
                            "ReduceScatter",
                            mybir.AluOpType.add,
                            replica_groups=RG,
                            ins=[tabk[k][:]],
                            outs=[outk[k][:]],
                        )

            # ---------------- Layer 1 aggregation ----------------
            if PHASE >= 2:
                agg_layer(tab1, agg1, hres, False)

            # ---------------- L1 post: relu, GEMM2, h2' ----------------
            for k in range(KCH if PHASE >= 4 else 0):
                rs1 = sb.tile([P, CH_ROWS], p1dt, tag="rs1", bufs=2,
                              name="rs1")
                nc.sync.dma_start(out=rs1[:], in_=agg1[k][:])
                v = sb.tile([P, CH_ROWS], p1dt, tag="v", bufs=2, name="v")
                nc.vector.tensor_tensor(
                    out=v[:], in0=rs1[:],
                    in1=dinvrow_sb[:, k * CH_ROWS:(k + 1) * CH_ROWS],
                    op=mybir.AluOpType.mult,
                )
                a1 = sb.tile([P, CH_ROWS], bf16, tag="a1", bufs=2, name="a1")
                nc.scalar.activation(
                    out=a1[:], in_=v[:],
                    func=mybir.ActivationFunctionType.Relu,
                    bias=b1_sb[:, 0:1],
                )
                for blk in range(BPC):
                    bis = k * BPC + blk
                    ps2 = psum.tile([P, NCLS_PAD], f32, tag="psum_2", bufs=2,
                                    name="ps_2")
                    nc.tensor.matmul(
                        out=ps2[:],
                        lhsT=a1[:, blk * P:(blk + 1) * P],
                        rhs=w2_sb[:],
                        start=True, stop=True,
                    )
                    nc.vector.tensor_scalar_mul(
                        out=h2res[:, bis * P:(bis + 1) * P],
                        in0=ps2[:],
                        scalar1=dinvn_sb[:, bis:bis + 1],
                    )
                nc.scalar.dma_start(
                    out=h2tab[k * CH_ROWS:(k + 1) * CH_ROWS, :].rearrange(
                        "(d b) h -> d (b h)", b=BPC),
                    in_=h2res[:, k * CH_ROWS:(k + 1) * CH_ROWS],
                )

            # ---------------- Layer 2 aggregation ----------------
            if PHASE >= 5:
                agg_layer(tab2, out2, h2res, True)

            # ---------------- L2 post: bias, log_softmax ----------------
            for k in range(KCH if PHASE >= 6 else 0):
                r2 = sb.tile([P, BPC * NCLS], p2dt, tag="r2", bufs=2,
                             name="r2")
                nc.sync.dma_start(
                    out=r2[:],
                    in_=out2[k][:].rearrange("(d b) n -> d (b n)", b=BPC),
                )
                lg = sb.tile([P, BPC * NCLS], f32, tag="lg", bufs=2, name="lg")
                nc.vector.tensor_tensor(
                    out=lg[:], in0=r2[:],
                    in1=dinvrep2_sb[:, k * BPC * NCLS:(k + 1) * BPC * NCLS],
                    op=mybir.AluOpType.mult,
                )
                nc.vector.tensor_tensor(
                    out=lg[:], in0=lg[:], in1=b2rep_sb[:],
                    op=mybir.AluOpType.add,
                )
                ostage = sb.tile([P, BPC * NCLS], f32, tag="ostage", bufs=2,
                                 name="ostage")
                for blk in range(BPC):
                    lgb = lg[:, blk * NCLS:(blk + 1) * NCLS]
                    negm = sb.tile([P, 1], f32, tag="negm", bufs=4,
                                   name="negm")
                    nc.vector.reduce_max(
                        out=negm[:], in_=lgb, axis=mybir.AxisListType.X
                    )
                    nc.vector.tensor_scalar_mul(
                        out=negm[:], in0=negm[:], scalar1=-1.0
                    )
                    expv = sb.tile([P, NCLS], f32, tag="expv", bufs=4,
                                   name="expv")
                    nc.scalar.activation(
                        out=expv[:], in_=lgb,
                        func=mybir.ActivationFunctionType.Exp,
                        bias=negm[:, 0:1],
                    )
                    ssum = sb.tile([P, 1], f32, tag="ssum", bufs=4,
                                   name="ssum")
                    nc.vector.reduce_sum(
                        out=ssum[:], in_=expv[:], axis=mybir.AxisListType.X
                    )
                    lns = sb.tile([P, 1], f32, tag="lns", bufs=4, name="lns")
                    nc.scalar.activation(
                        out=lns[:], in_=ssum[:],
                        func=mybir.ActivationFunctionType.Ln,
                    )
                    nc.vector.tensor_scalar(
                        out=ostage[:, blk * NCLS:(blk + 1) * NCLS],
                        in0=lgb,
                        scalar1=negm[:, 0:1], scalar2=lns[:, 0:1],
                        op0=mybir.AluOpType.add, op1=mybir.AluOpType.subtract,
                    )
                nc.scalar.dma_start(
                    out=out_d[k * CH_ROWS:(k + 1) * CH_ROWS, :].rearrange(
                        "(d b) n -> d (b n)", b=BPC),
                    in_=ostage[:],
                )

    nc.compile()
    return nc


# --------------------------------------------------------------------------
# Host orchestration
# --------------------------------------------------------------------------

def _make_in_maps(x, W1, b1, W2, b2, newid, dinv, idx_all, seg_all, T2):
    x = np.asarray(x, np.float32)
    W1 = np.asarray(W1, np.float32)
    W2 = np.asarray(W2, np.float32)
    b1v = np.asarray(b1, np.float32).reshape(-1)
    b2v = np.asarray(b2, np.float32).reshape(-1)
    kt = F_IN // P
    tnp = _tab_np()

    x_new = np.zeros((NPAD, F_IN), np.float32)
    x_new[newid] = x

    w1r = np.ascontiguousarray(
        W1.reshape(kt, P, HID).transpose(1, 0, 2).reshape(P, kt * HID)
    ).astype(tnp)
    w2p = np.zeros((HID, NCLS_PAD), np.float32)
    w2p[:, :NCLS] = W2
    w2p = w2p.astype(tnp)
    b2rep = np.ascontiguousarray(
        np.broadcast_to(np.tile(b2v, BPC), (P, BPC * NCLS))
    ).astype(np.float32)

    gt = GR * T2
    iotaw = np.ascontiguousarray(
        np.broadcast_to(np.tile(np.arange(P, dtype=np.float32), gt),
                        (P, gt * P))
    ).astype(tnp)

    p1np = _np_pdt(PART_DT)
    p2np = _np_pdt(PART2_DT)

    in_maps = []
    for c in range(NC):
        xt_c = np.ascontiguousarray(
            x_new[c * SH:(c + 1) * SH].T
        ).astype(tnp)
        dv = dinv[c * SH:(c + 1) * SH]
        dinvn = np.ascontiguousarray(
            dv.reshape(NBPS, P).T
        ).astype(np.float32)
        dinvrow = np.ascontiguousarray(
            np.broadcast_to(dv, (P, SH))
        ).astype(p1np)
        # dinvrep2: col bis*40 + n, row p=d -> dinv[node bis*128+p]
        dinvrep2 = np.ascontiguousarray(np.repeat(
            dv.reshape(NBPS, P).T, NCLS, axis=1
        )).astype(p2np)
        eyem = np.zeros((P, NC * P), np.float32)
        eyem[:, c * P:(c + 1) * P] = np.eye(P, dtype=np.float32)
        in_maps.append({
            "xt": xt_c,
            "w1": w1r,
            "w2": w2p,
            "b1": b1v.reshape(P, 1).astype(np.float32).copy(),
            "b2rep": b2rep,
            "dinvn": dinvn,
            "dinvrow": dinvrow,
            "dinvrep2": dinvrep2,
            "eyem": eyem.astype(tnp),
            "iotaw": iotaw,
            "seg": np.ascontiguousarray(seg_all[c]).astype(tnp),
            "idx": np.ascontiguousarray(idx_all[c]),
        })
    return in_maps


def kernel(x, edge_index, W1, b1, W2, b2):
    from concourse.bass_utils import run_bass_kernel_spmd

    global LAST_RESULT

    newid, dinv, idx_all, seg_all, T2 = _preprocess(edge_index)
    nc = _build_program(T2)
    in_maps = _make_in_maps(x, W1, b1, W2, b2, newid, dinv,
                            idx_all, seg_all, T2)

    res = run_bass_kernel_spmd(
        nc, in_maps, core_ids=list(range(NC)),
        trace=TRACE, trace_kwargs=dict(TRACE_KWARGS),
    )
    LAST_RESULT = {
        "exec_time_ns": res.exec_time_ns,
        "mean_exec_time_ns": res.mean_exec_time_ns,
        "instructions_and_trace": res.instructions_and_trace,
        "profile_json": res.profile_json,
        "T": T2,
        "nc": nc,
        "in_maps": in_maps,
        "newid": newid,
    }
    return assemble(newid, [r["out"] for r in res.results])


def assemble(newid, per_core_out):
    # device row r (per core): k=r//896, q=r%896, d=q//7, blk=q%7
    # -> local node n = (k*7+blk)*128 + d
    r = np.arange(SH)
    k = r // CH_ROWS
    q = r % CH_ROWS
    d = q // BPC
    blk = q % BPC
    n_loc = (k * BPC + blk) * P + d
    out_new = np.empty((NPAD, NCLS), np.float32)
    for c in range(NC):
        out_new[c * SH + n_loc] = per_core_out[c]
    return out_new[newid]


# revision 16
# speedup vs baseline: 1.1451x; 1.1451x over previous
"""GCN (2-layer, GCNConv + log_softmax) on 8 Trainium2 NeuronCores.

Strategy (1D node partition, per sharding hint):
  - Nodes padded to N_PAD = 392*128 and sharded contiguously: 49 blocks of 128
    dst-nodes per core.
  - CPU preprocessing: add self-loops, compute symmetric norm, sort edges by
    dst, pack per (core, block) into fixed-size edge tiles of 128 (padded with
    norm=0 edges so all cores run an identical instruction stream).
  - On device per core:
      GEMM1: h = x_shard @ W1 (PE, bf16 operands, fp32 accum)
      AllGather h -> full h table in local HBM
      Agg1 per dst block: indirect-DMA gather h[src] for all edge tiles of the
        block, build scaled selector S[e,dst] = (seg[e]==dst)*norm[e] on DVE,
        segment-sum via PE matmul accumulation into PSUM [hid, dst];
        relu(agg+b1) on ACT; fused GEMM2 -> h2 block; store to h2 shard.
      AllGather h2 -> full h2 table
      Agg2 per dst block: gather h2[src], same selector, accumulate [dst, cls];
        +b2, log_softmax on DVE/ACT; store output shard.
  - Host concatenates the 8 output shards and strips padding.
"""

import math

import numpy as np
import ml_dtypes

P = 128
NCORES = 8

# Full-problem constants (hardcoded per harness contract).
N_NODES = 50000
N_EDGES = 800000
F_IN = 512
HIDDEN = 128
N_CLASSES = 40

# Runtime-tunable knobs (test.py may override before calling kernel()).
TRACE = False
TRACE_KWARGS = {}
H_DTYPE = "bfloat16"    # dtype of the h (layer-1 projected) gather table
H2_DTYPE = "float32"    # dtype of the h2 (layer-2 projected) gather table
X_DTYPE = "bfloat16"    # GEMM1 operand dtype

LAST_RESULT = {}        # test.py introspection (exec time etc.)


def _np_dt(name):
    return {"float32": np.float32, "bfloat16": ml_dtypes.bfloat16}[name]


# --------------------------------------------------------------------------
# CPU preprocessing
# --------------------------------------------------------------------------

def _preprocess(edge_index, n_nodes, blocks_per_core):
    """Sort edges (plus self-loops) by dst, pack into fixed-count edge tiles.

    Returns (srcs, segs, norms, T):
      srcs  [NCORES, 128, BPC*T] int32   src node id of edge p in tile g
      segs  [NCORES, 128, BPC*T] float32 dst % 128 (local row in block)
      norms [NCORES, 128, BPC*T] float32 dinv[src]*dinv[dst] (0 for padding)
      T = edge tiles per block (uniform across all cores/blocks)
    """
    nblk = NCORES * blocks_per_core
    src = np.asarray(edge_index[0], dtype=np.int64)
    dst = np.asarray(edge_index[1], dtype=np.int64)

    deg = np.bincount(dst, minlength=n_nodes).astype(np.float32) + 1.0
    dinv = (1.0 / np.sqrt(deg)).astype(np.float32)

    loops = np.arange(n_nodes, dtype=np.int64)
    all_src = np.concatenate([src, loops])
    all_dst = np.concatenate([dst, loops])
    norm = dinv[all_src] * dinv[all_dst]

    order = np.argsort(all_dst, kind="stable")
    s_src = all_src[order].astype(np.int32)
    s_dst = all_dst[order]
    s_norm = norm[order].astype(np.float32)

    blk = s_dst // P
    seg = (s_dst % P).astype(np.float32)
    counts = np.bincount(blk, minlength=nblk)
    T = max(1, int(math.ceil(counts.max() / P)))

    nt = blocks_per_core * T
    srcs = np.zeros((NCORES, P, nt), np.int32)
    segs = np.zeros((NCORES, P, nt), np.float32)
    norms = np.zeros((NCORES, P, nt), np.float32)

    starts = np.concatenate([[0], np.cumsum(counts)])
    for b in range(nblk):
        c, bl = divmod(b, blocks_per_core)
        lo, hi = int(starts[b]), int(starts[b + 1])
        n = hi - lo
        if n == 0:
            continue
        j = np.arange(n)
        g = bl * T + j // P
        p = j % P
        srcs[c, p, g] = s_src[lo:hi]
        segs[c, p, g] = seg[lo:hi]
        norms[c, p, g] = s_norm[lo:hi]
    return srcs, segs, norms, T


# --------------------------------------------------------------------------
# Device program
# --------------------------------------------------------------------------

def _build_program(f_in, hidden, ncls_pad, blocks_per_core, T, hdt_name,
                   h2dt_name, xdt_name):
    import concourse.bacc as bacc
    import concourse.bass as bass
    import concourse.mybir as mybir
    import concourse.tile as tile

    dt = mybir.dt
    name2dt = {"float32": dt.float32, "bfloat16": dt.bfloat16}
    hdt = name2dt[hdt_name]
    h2dt = name2dt[h2dt_name]
    xdt = name2dt[xdt_name]
    f32 = dt.float32

    shard = blocks_per_core * P
    n_pad = NCORES * shard
    nt = blocks_per_core * T
    kt = f_in // P  # k-tiles in GEMM1

    nc = bacc.Bacc(
        "TRN2",
        target_bir_lowering=False,
        debug=False,
        enable_asserts=False,
        num_devices=NCORES,
    )

    # Kernel I/O
    xt_d = nc.dram_tensor("xt", [f_in, shard], xdt, kind="ExternalInput")
    w1_d = nc.dram_tensor("w1", [P, kt * hidden], xdt, kind="ExternalInput")
    b1_d = nc.dram_tensor("b1", [P, 1], f32, kind="ExternalInput")
    w2_d = nc.dram_tensor("w2", [hidden, ncls_pad], f32, kind="ExternalInput")
    b2_d = nc.dram_tensor("b2t", [P, ncls_pad], f32, kind="ExternalInput")
    iota_d = nc.dram_tensor("iotaw", [P, T * P], f32, kind="ExternalInput")
    srcs_d = nc.dram_tensor("srcs", [P, nt], dt.int32, kind="ExternalInput")
    segs_d = nc.dram_tensor("segs", [P, nt], f32, kind="ExternalInput")
    norms_d = nc.dram_tensor("norms", [P, nt], f32, kind="ExternalInput")
    out_d = nc.dram_tensor("out", [shard, N_CLASSES], f32, kind="ExternalOutput")

    RG = [list(range(NCORES))]

    with tile.TileContext(nc) as tc:
        with (
            tc.tile_pool(name="const", bufs=1) as const,
            tc.tile_pool(name="dram", bufs=1, space="DRAM") as dram,
            tc.tile_pool(name="sb", bufs=3) as sb,
            tc.tile_pool(name="psum", bufs=2, space="PSUM") as psum,
        ):
            # Internal DRAM buffers
            h_ag_in = dram.tile([shard, hidden], hdt)
            h_full = dram.tile([n_pad, hidden], hdt, addr_space="Shared")
            h2_ag_in = dram.tile([shard, ncls_pad], h2dt)
            h2_full = dram.tile([n_pad, ncls_pad], h2dt, addr_space="Shared")

            # Constants into SBUF
            w1_sb = const.tile([P, kt * hidden], xdt)
            nc.sync.dma_start(out=w1_sb[:], in_=w1_d[:])
            b1_sb = const.tile([P, 1], f32)
            nc.sync.dma_start(out=b1_sb[:], in_=b1_d[:])
            w2_sb = const.tile([hidden, ncls_pad], f32)
            nc.sync.dma_start(out=w2_sb[:], in_=w2_d[:])
            b2_sb = const.tile([P, ncls_pad], f32)
            nc.sync.dma_start(out=b2_sb[:], in_=b2_d[:])
            iota_sb = const.tile([P, T * P], f32)
            nc.sync.dma_start(out=iota_sb[:], in_=iota_d[:])
            srcs_sb = const.tile([P, nt], dt.int32)
            nc.sync.dma_start(out=srcs_sb[:], in_=srcs_d[:])
            segs_sb = const.tile([P, nt], f32)
            nc.sync.dma_start(out=segs_sb[:], in_=segs_d[:])
            norms_sb = const.tile([P, nt], f32)
            nc.sync.dma_start(out=norms_sb[:], in_=norms_d[:])

            # ---------------- Phase 1: GEMM1 (h = x @ W1) ----------------
            for i in range(blocks_per_core):
                psum_h = psum.tile([P, hidden], f32, tag="psum_h")
                for k in range(kt):
                    xt_t = sb.tile([P, P], xdt, tag="xt", bufs=4)
                    nc.sync.dma_start(
                        out=xt_t[:],
                        in_=xt_d[k * P:(k + 1) * P, i * P:(i + 1) * P],
                    )
                    nc.tensor.matmul(
                        out=psum_h[:],
                        lhsT=xt_t[:],
                        rhs=w1_sb[:, k * hidden:(k + 1) * hidden],
                        start=(k == 0),
                        stop=(k == kt - 1),
                    )
                h_t = sb.tile([P, hidden], hdt, tag="h_t")
                nc.vector.tensor_copy(out=h_t[:], in_=psum_h[:])
                nc.sync.dma_start(
                    out=h_ag_in[i * P:(i + 1) * P, :], in_=h_t[:]
                )

            # ---------------- AllGather h ----------------
            nc.gpsimd.collective_compute(
                "AllGather",
                mybir.AluOpType.bypass,
                replica_groups=RG,
                ins=[h_ag_in[:]],
                outs=[h_full[:]],
            )

            # ---------------- Phase 2: Agg1 + relu + GEMM2 ----------------
            def build_selector(b, seldt):
                g0 = b * T
                sel = sb.tile([P, T * P], seldt, tag="sel")
                sel3 = sel[:].rearrange("p (t d) -> p t d", d=P)
                nc.vector.tensor_tensor(
                    out=sel3,
                    in0=iota_sb[:].rearrange("p (t d) -> p t d", d=P),
                    in1=segs_sb[:, g0:g0 + T].to_broadcast([P, T, P]),
                    op=mybir.AluOpType.is_equal,
                )
                nc.vector.tensor_tensor(
                    out=sel3,
                    in0=sel3,
                    in1=norms_sb[:, g0:g0 + T].to_broadcast([P, T, P]),
                    op=mybir.AluOpType.mult,
                )
                return sel

            for b in range(blocks_per_core):
                g0 = b * T
                msg = sb.tile([P, T * hidden], hdt, tag="msg")
                for t in range(T):
                    nc.gpsimd.indirect_dma_start(
                        out=msg[:, t * hidden:(t + 1) * hidden],
                        out_offset=None,
                        in_=h_full[:],
                        in_offset=bass.IndirectOffsetOnAxis(
                            ap=srcs_sb[:, g0 + t:g0 + t + 1], axis=0
                        ),
                    )
                sel = build_selector(b, hdt)
                psum1 = psum.tile([P, P], f32, tag="psum1")
                for t in range(T):
                    nc.tensor.matmul(
                        out=psum1[:],
                        lhsT=msg[:, t * hidden:(t + 1) * hidden],
                        rhs=sel[:, t * P:(t + 1) * P],
                        start=(t == 0),
                        stop=(t == T - 1),
                    )
                # psum1 = agg1^T : [hidden, dst]; relu(agg + b1) with b1 along
                # partitions.
                a1 = sb.tile([P, P], f32, tag="a1")
                nc.scalar.activation(
                    out=a1[:], in_=psum1[:],
                    func=mybir.ActivationFunctionType.Relu,
                    bias=b1_sb[:, 0:1],
                )
                psum2 = psum.tile([P, ncls_pad], f32, tag="psum2")
                nc.tensor.matmul(
                    out=psum2[:], lhsT=a1[:], rhs=w2_sb[:],
                    start=True, stop=True,
                )
                h2_t = sb.tile([P, ncls_pad], h2dt, tag="h2_t")
                nc.vector.tensor_copy(out=h2_t[:], in_=psum2[:])
                nc.sync.dma_start(
                    out=h2_ag_in[b * P:(b + 1) * P, :], in_=h2_t[:]
                )

            # ---------------- AllGather h2 ----------------
            nc.gpsimd.collective_compute(
                "AllGather",
                mybir.AluOpType.bypass,
                replica_groups=RG,
                ins=[h2_ag_in[:]],
                outs=[h2_full[:]],
            )

            # ---------------- Phase 3: Agg2 + bias + log_softmax ----------------
            for b in range(blocks_per_core):
                g0 = b * T
                msg2 = sb.tile([P, T * ncls_pad], h2dt, tag="msg2")
                for t in range(T):
                    nc.gpsimd.indirect_dma_start(
                        out=msg2[:, t * ncls_pad:(t + 1) * ncls_pad],
                        out_offset=None,
                        in_=h2_full[:],
                        in_offset=bass.IndirectOffsetOnAxis(
                            ap=srcs_sb[:, g0 + t:g0 + t + 1], axis=0
                        ),
                    )
                sel = build_selector(b, h2dt)
                psum_o = psum.tile([P, ncls_pad], f32, tag="psum_o")
                for t in range(T):
                    nc.tensor.matmul(
                        out=psum_o[:],
                        lhsT=sel[:, t * P:(t + 1) * P],
                        rhs=msg2[:, t * ncls_pad:(t + 1) * ncls_pad],
                        start=(t == 0),
                        stop=(t == T - 1),
                    )
                logits = sb.tile([P, N_CLASSES], f32, tag="logits")
                nc.vector.tensor_tensor(
                    out=logits[:], in0=psum_o[:, 0:N_CLASSES],
                    in1=b2_sb[:, 0:N_CLASSES], op=mybir.AluOpType.add,
                )
                negm = sb.tile([P, 1], f32, tag="negm")
                nc.vector.reduce_max(
                    out=negm[:], in_=logits[:], axis=mybir.AxisListType.X
                )
                nc.vector.tensor_scalar_mul(
                    out=negm[:], in0=negm[:], scalar1=-1.0
                )
                expv = sb.tile([P, N_CLASSES], f32, tag="expv")
                nc.scalar.activation(
                    out=expv[:], in_=logits[:],
                    func=mybir.ActivationFunctionType.Exp,
                    bias=negm[:, 0:1],
                )
                ssum = sb.tile([P, 1], f32, tag="ssum")
                nc.vector.reduce_sum(
                    out=ssum[:], in_=expv[:], axis=mybir.AxisListType.X
                )
                lns = sb.tile([P, 1], f32, tag="lns")
                nc.scalar.activation(
                    out=lns[:], in_=ssum[:],
                    func=mybir.ActivationFunctionType.Ln,
                )
                outt = sb.tile([P, N_CLASSES], f32, tag="outt")
                nc.vector.tensor_scalar(
                    out=outt[:], in0=logits[:],
                    scalar1=negm[:, 0:1], scalar2=lns[:, 0:1],
                    op0=mybir.AluOpType.add, op1=mybir.AluOpType.subtract,
                )
                nc.sync.dma_start(
                    out=out_d[b * P:(b + 1) * P, :], in_=outt[:]
                )

    nc.compile()
    return nc


# --------------------------------------------------------------------------
# Host orchestration
# --------------------------------------------------------------------------

def _run(x, edge_index, W1, b1, W2, b2, blocks_per_core):
    from concourse.bass_utils import run_bass_kernel_spmd

    global LAST_RESULT

    x = np.asarray(x, dtype=np.float32)
    W1 = np.asarray(W1, dtype=np.float32)
    b1v = np.asarray(b1, dtype=np.float32).reshape(-1)
    W2 = np.asarray(W2, dtype=np.float32)
    b2v = np.asarray(b2, dtype=np.float32).reshape(-1)

    n_nodes, f_in = x.shape
    hidden = W1.shape[1]
    ncls = W2.shape[1]
    ncls_pad = 64 if ncls <= 64 else int(math.ceil(ncls / P) * P)
    assert hidden == P and ncls == N_CLASSES

    shard = blocks_per_core * P
    n_pad = NCORES * shard
    assert n_pad >= n_nodes

    srcs, segs, norms, T = _preprocess(edge_index, n_nodes, blocks_per_core)

    nc = _build_program(
        f_in, hidden, ncls_pad, blocks_per_core, T,
        H_DTYPE, H2_DTYPE, X_DTYPE,
    )

    xdt_np = _np_dt(X_DTYPE)
    kt = f_in // P

    x_pad = np.zeros((n_pad, f_in), np.float32)
    x_pad[:n_nodes] = x
    w1r = np.ascontiguousarray(
        W1.reshape(kt, P, hidden).transpose(1, 0, 2).reshape(P, kt * hidden)
    ).astype(xdt_np)
    w2p = np.zeros((hidden, ncls_pad), np.float32)
    w2p[:, :ncls] = W2
    b2t = np.zeros((P, ncls_pad), np.float32)
    b2t[:, :ncls] = b2v[None, :]
    iotaw = np.ascontiguousarray(
        np.broadcast_to(
            np.tile(np.arange(P, dtype=np.float32), T), (P, T * P)
        )
    )

    in_maps = []
    for c in range(NCORES):
        xt_c = np.ascontiguousarray(
            x_pad[c * shard:(c + 1) * shard].T
        ).astype(xdt_np)
        in_maps.append({
            "xt": xt_c,
            "w1": w1r,
            "b1": b1v.reshape(P, 1).copy(),
            "w2": w2p,
            "b2t": b2t,
            "iotaw": iotaw,
            "srcs": np.ascontiguousarray(srcs[c]),
            "segs": np.ascontiguousarray(segs[c]),
            "norms": np.ascontiguousarray(norms[c]),
        })

    res = run_bass_kernel_spmd(
        nc, in_maps, core_ids=list(range(NCORES)),
        trace=TRACE, trace_kwargs=dict(TRACE_KWARGS),
    )
    LAST_RESULT = {
        "exec_time_ns": res.exec_time_ns,
        "mean_exec_time_ns": res.mean_exec_time_ns,
        "instructions_and_trace": res.instructions_and_trace,
        "profile_json": res.profile_json,
        "T": T,
        "nc": nc,
        "in_maps": in_maps,
    }
    out = np.concatenate([r["out"] for r in res.results], axis=0)
    return out[:n_nodes]


def kernel(x, edge_index, W1, b1, W2, b2):
    n_nodes = np.asarray(x).shape[0]
    blocks_per_core = int(math.ceil(n_nodes / (NCORES * P)))
    return _run(x, edge_index, W1, b1, W2, b2, blocks_per_core)

